# revision 20
# baseline (speedup 1.0000x reference)
"""Multi-head attention (B=2, N=2048, D=1024, H=16, hd=64) on 8 TRN2 NeuronCores.

Sharding: data-parallel over batch (2) x tensor-parallel over heads (4 groups
of 4 heads). Each core computes, for its (batch b, head group g), the partial
output over its 256 sharded head dims; the host sums the 4 head-group
partials per batch, adds bo.

Per-core device kernel (all matmuls bf16 with fp32 PSUM accumulation):
  Phase A: xT/weights streamed over 3 DMA queues; Q^T/K^T (zero-padded
  two-heads-per-tile layout) and V|ones projections.
  Phase B: 4 windows (i-half x head-pair).  Per jb: S^T matmuls -> ACT exp
  (PSUM f32 -> SBUF bf16) -> PV accumulation (V|ones stationary gives
  Z = sum_j exp in PSUM row 64).  The output projection for i-half 0 is
  absorbed into window 3's jb loop (PV deferred to the window's back half so
  the o-tag PSUM banks are free early); the o=0 half of i-half 1's output
  projection is absorbed into window 4 the same way (host sums the o-split
  partials), leaving only the o=1 half on the tail.  Normalization uses
  reciprocal_approx_fast + gpsimd partition_broadcast (attn library); the
  pair-1 windows multiply PSUM directly to shorten the critical chain.
"""
import sys

sys.path.insert(0, "/opt/trn_rl_repo")

import numpy as np
import ml_dtypes

import concourse.bass as bass
import concourse.tile as tile
from concourse import bacc, bass_utils, mybir, library_config

P = 128
NTOK = 2048          # sequence length
D = 1024             # model dim
HPC = 4              # heads per core
HD = 64              # head dim
DSH = HPC * HD       # 256: sharded head dims per core
CO = D // P          # 8 contraction chunks over c
NIH = 2              # i halves
IHW = NTOK // NIH    # 1024
NC2 = IHW // 512     # 512-chunks per half = 2
NJB = NTOK // P      # 16 j blocks
SCALE = HD ** -0.5

F32 = mybir.dt.float32
BF16 = mybir.dt.bfloat16
DT_NP = np.float32
BF16_NP = ml_dtypes.bfloat16


def build_nc():
    nc = bacc.Bacc("TRN2", target_bir_lowering=False, debug=False)

    xt_d = nc.dram_tensor("xt", [D, NTOK], BF16, kind="ExternalInput").ap()
    wqt_d = nc.dram_tensor("wqt", [D, DSH], BF16, kind="ExternalInput").ap()
    wkt_d = nc.dram_tensor("wkt", [D, DSH], BF16, kind="ExternalInput").ap()
    wvt_d = nc.dram_tensor("wvt", [D, DSH], BF16, kind="ExternalInput").ap()
    wot_d = nc.dram_tensor("wot", [DSH, D], BF16, kind="ExternalInput").ap()
    outt_d = nc.dram_tensor("outt", [D, NTOK], BF16, kind="ExternalOutput").ap()
    out0_d = nc.dram_tensor("out0", [D, IHW], BF16, kind="ExternalOutput").ap()

    xt_t = xt_d.rearrange("(o p) i -> p o i", p=P)        # [128, 8, 2048]
    wq_t = wqt_d.rearrange("(o p) d -> p o d", p=P)       # [128, 8, 256]
    wk_t = wkt_d.rearrange("(o p) d -> p o d", p=P)
    wv_t = wvt_d.rearrange("(o p) d -> p o d", p=P)
    wo_t = wot_d.rearrange("(o p) e -> p o e", p=P)       # [128, 2, 1024]
    out_t = outt_d.rearrange("(m p) i -> p m i", p=P)     # [128, 8, 2048]
    out0_t = out0_d.rearrange("(m p) i -> p m i", p=P)    # [128, 8, 1024]

    with tile.TileContext(nc) as tc:
        with (
            tc.tile_pool(name="sbp", bufs=1) as sbp,           # persistent
            tc.tile_pool(name="ps", bufs=1, space="PSUM") as ps,
        ):
            # persistent activation tensors
            qt = sbp.tile([P, 2, NTOK], BF16, tag="qt")        # Q^T natural
            ktp = sbp.tile([P, HPC, NTOK], BF16, tag="ktp")    # K^T padded
            vaug = sbp.tile([P, NJB, HPC, 65], BF16, tag="vaug")  # V | ones
            ota = sbp.tile([P, 2, NTOK], BF16, tag="ota")      # O^T all heads
            wo = sbp.tile([P, 2, D], BF16, tag="wo")

            nc.gpsimd.load_library(library_config.attn)

            # zero rows of the padded K^T slots; ones column of vaug (DVE)
            nc.vector.memset(ktp[64:128, 0, :], 0.0)
            nc.vector.memset(ktp[0:64, 1, :], 0.0)
            nc.vector.memset(ktp[64:128, 2, :], 0.0)
            nc.vector.memset(ktp[0:64, 3, :], 0.0)
            nc.vector.memset(vaug[:, :, :, 64:65], 1.0)

            # psum tags: a0/a1 [128,1024] (2 banks each), o0..o3 (1 bank each)
            def psA(i):
                return ps.tile([P, IHW], F32, tag=f"a{i % 2}", name=f"psA{i % 2}")

            _ot_ctr = [0]

            def psO():
                i = _ot_ctr[0]
                _ot_ctr[0] += 1
                return ps.tile([P, 512], F32, tag=f"o{i % 4}", name=f"psO{i % 4}")

            # ------------- phase A: loads + projections (xt/w scoped) -------
            with tc.tile_pool(name="sbl", bufs=1) as sbl:
                wq = sbl.tile([P, CO, DSH], BF16, tag="wq")
                wk = sbl.tile([P, CO, DSH], BF16, tag="wk")
                wv = sbl.tile([P, CO, DSH], BF16, tag="wv")
                xt = [sbl.tile([P, NTOK], BF16, tag=f"xt{o}", name=f"xt{o}")
                      for o in range(CO)]
                # spread input loads over the two hardware DMA queues;
                # mt0 halves of Wq/Wk first so A1 can start immediately
                nc.scalar.dma_start(wq[:, :, 0:P], wq_t[:, :, 0:P])
                nc.sync.dma_start(wk[:, :, 0:P], wk_t[:, :, 0:P])
                for o in range(CO):
                    eng = nc.sync if o % 2 == 0 else nc.scalar
                    eng.dma_start(xt[o][:], xt_t[:, o, :])
                nc.scalar.dma_start(wq[:, :, P:DSH], wq_t[:, :, P:DSH])
                nc.sync.dma_start(wk[:, :, P:DSH], wk_t[:, :, P:DSH])
                nc.scalar.dma_start(wv[:], wv_t)
                nc.sync.dma_start(wo[:], wo_t)

                # A1: Q(mt0) + K(mt0), o-streamed so the PE starts on the
                # first xT chunk.  Q groups [128,1024] on a-tags; K groups
                # [128,512] per (ih,c) on o-tags.
                ppq = {ih: psA(ih) for ih in range(NIH)}
                ppk = {(ih, c): psO() for ih in range(NIH) for c in range(NC2)}
                for o in range(CO):
                    for ih in range(NIH):
                        for c in range(NC2):
                            nc.tensor.matmul(
                                ppq[ih][:, c * 512:(c + 1) * 512],
                                wq[:, o, 0:P],
                                xt[o][:, ih * IHW + c * 512:
                                      ih * IHW + (c + 1) * 512],
                                start=(o == 0), stop=(o == CO - 1),
                            )
                            nc.tensor.matmul(
                                ppk[(ih, c)][:],
                                wk[:, o, 0:P],
                                xt[o][:, ih * IHW + c * 512:
                                      ih * IHW + (c + 1) * 512],
                                start=(o == 0), stop=(o == CO - 1),
                            )
                for ih in range(NIH):
                    sl = slice(ih * IHW, (ih + 1) * IHW)
                    nc.scalar.copy(qt[:, 0, sl], ppq[ih][:, :])
                    for c in range(NC2):
                        slc = slice(ih * IHW + c * 512, ih * IHW + (c + 1) * 512)
                        nc.scalar.copy(ktp[0:64, 0, slc], ppk[(ih, c)][0:64, :])
                        nc.scalar.copy(ktp[64:128, 1, slc], ppk[(ih, c)][64:128, :])

                # A2: Q(mt1) / K(mt1) (all xT resident by now), then V
                for w_sb, is_q in ((wq, True), (wk, False)):
                    for ih in range(NIH):
                        pp = psA(ih)
                        for c in range(NC2):
                            for o in range(CO):
                                nc.tensor.matmul(
                                    pp[:, c * 512:(c + 1) * 512],
                                    w_sb[:, o, P:2 * P],
                                    xt[o][:, ih * IHW + c * 512:
                                          ih * IHW + (c + 1) * 512],
                                    start=(o == 0), stop=(o == CO - 1),
                                )
                        sl = slice(ih * IHW, (ih + 1) * IHW)
                        if is_q:
                            nc.scalar.copy(qt[:, 1, sl], pp[:, :])
                        else:
                            nc.scalar.copy(ktp[0:64, 2, sl], pp[0:64, :])
                            nc.scalar.copy(ktp[64:128, 3, sl], pp[64:128, :])

                # V projection
                for it in range(NJB):
                    pv = psO()
                    for o in range(CO):
                        nc.tensor.matmul(
                            pv[:, 0:DSH],
                            xt[o][:, it * P:(it + 1) * P],
                            wv[:, o, :],
                            start=(o == 0), stop=(o == CO - 1),
                        )
                    nc.scalar.copy(
                        vaug[:, it, :, 0:64],
                        pv[:, 0:DSH].rearrange("p (h d) -> p h d", d=HD),
                    )

            # ------------- phase B: attention + output projection -----------
            with tc.tile_pool(name="sbw", bufs=1) as sbw:
                ES_BUFS = 24

                def op_unit(ih, mt, c, o_list, dest, evac="v"):
                    """One output-projection unit: psum [128,512], matmuls
                    over o_list, evacuate bf16, DMA out."""
                    pe = psO()
                    n_o = len(o_list)
                    for i, o in enumerate(o_list):
                        nc.tensor.matmul(
                            pe[:],
                            wo[:, o, mt * P:(mt + 1) * P],
                            ota[:, o, ih * IHW + c * 512:
                                ih * IHW + (c + 1) * 512],
                            start=(i == 0), stop=(i == n_o - 1),
                        )
                    stg = sbw.tile([P, 512], BF16, tag="stg", bufs=4)
                    if evac == "v":
                        nc.vector.tensor_copy(stg[:], pe[:])
                    else:
                        nc.scalar.copy(stg[:], pe[:])
                    if dest == 0:
                        nc.sync.dma_start(
                            out_t[:, mt, ih * IHW + c * 512:
                                  ih * IHW + (c + 1) * 512], stg[:])
                    else:
                        nc.scalar.dma_start(
                            out0_t[:, mt, c * 512:(c + 1) * 512], stg[:])

                def normalize(ih, pair, heads, pso, direct):
                    """Scale O^T rows by 1/Z.  direct=True multiplies PSUM
                    in place (shorter chain, holds the o-tag banks); False
                    evacuates first (frees banks for the next window's PV).
                    The chain is spread across ACT (z-copies), DVE (recip,
                    mults) and Pool (broadcasts), c-major so the c0 outputs
                    finish first for downstream c-major consumers."""
                    ot, rb, rts = {}, {}, {}
                    for k, h in enumerate(heads):
                        zt = sbw.tile([1, IHW], F32, tag="zt", bufs=2)
                        for c in range(NC2):
                            nc.scalar.copy(zt[:, c * 512:(c + 1) * 512],
                                           pso[(h, c)][64:65, :])
                        rts[h] = sbw.tile([1, IHW], F32, tag="rt", bufs=2,
                                          name="rt")
                        nc.vector.reciprocal_approx_fast(out=rts[h][:],
                                                         in_=zt[:])
                        rb[h] = sbw.tile([64, IHW], F32, tag="rb", bufs=2,
                                         name="rb")
                        if not direct:
                            ot[h] = sbw.tile([64, IHW], F32, tag="otmp",
                                             bufs=2, name="otmp")
                            for c in range(NC2):
                                nc.vector.tensor_copy(
                                    ot[h][:, c * 512:(c + 1) * 512],
                                    pso[(h, c)][0:64, :])
                    for c in range(NC2):
                        for k, h in enumerate(heads):
                            nc.gpsimd.partition_broadcast(
                                rb[h][:, c * 512:(c + 1) * 512],
                                rts[h][:, c * 512:(c + 1) * 512])
                    for c in range(NC2):
                        for k, h in enumerate(heads):
                            row = slice((h % 2) * 64, (h % 2) * 64 + 64)
                            src = (pso[(h, c)][0:64, :] if direct
                                   else ot[h][:, c * 512:(c + 1) * 512])
                            nc.vector.tensor_mul(
                                ota[row, h // 2, ih * IHW + c * 512:
                                    ih * IHW + (c + 1) * 512],
                                src,
                                rb[h][:, c * 512:(c + 1) * 512],
                            )

                for wi, (ih, pair) in enumerate([(0, 0), (0, 1), (1, 0), (1, 1)]):
                    heads = (2 * pair, 2 * pair + 1)
                    pso = {}

                    def get_pso(h, c):
                        if (h, c) not in pso:
                            pso[(h, c)] = psO()
                        return pso[(h, c)]

                    def emit_S(jb):
                        pst = {}
                        for k, h in enumerate(heads):
                            pa = psA(k)
                            pst[h] = pa
                            for c in range(NC2):
                                nc.tensor.matmul(
                                    pa[:, c * 512:(c + 1) * 512],
                                    ktp[:, h, jb * P:(jb + 1) * P],
                                    qt[:, h // 2, ih * IHW + c * 512:
                                       ih * IHW + (c + 1) * 512],
                                    start=True, stop=True,
                                )
                        return pst

                    def emit_exp(pst):
                        ess = {}
                        for k, h in enumerate(heads):
                            es = sbw.tile([P, IHW], BF16, tag="es",
                                          bufs=ES_BUFS)
                            nc.scalar.activation(
                                es[:], pst[h][:],
                                mybir.ActivationFunctionType.Exp,
                            )
                            ess[h] = es
                        return ess

                    def emit_PV(jb, ess):
                        for k, h in enumerate(heads):
                            for c in range(NC2):
                                nc.tensor.matmul(
                                    get_pso(h, c)[0:65, :],
                                    vaug[:, jb, h, 0:65],
                                    ess[h][:, c * 512:(c + 1) * 512],
                                    start=(jb == 0), stop=(jb == NJB - 1),
                                )

                    # per-jb schedules: (op_units_emitted_this_jb, pvs)
                    if wi < 2:
                        op_sched = {}
                        pv_sched = {jb: [jb - 1] for jb in range(1, NJB)}
                        pv_sched[NJB] = [NJB - 1]
                    elif wi == 2:
                        # absorb OP(ih=0) full units at jb2..9 (after the
                        # previous window's normalize chain); PV deferred
                        units = [(0, mt, c, [0, 1], 0)
                                 for mt in range(8) for c in range(NC2)]
                        op_sched = {jb: units[2 * (jb - 2):2 * (jb - 2) + 2]
                                    for jb in range(2, 10)}
                        pv_sched = {10: [0, 1], 11: [2, 3], 12: [4, 5, 6],
                                    13: [7, 8, 9], 14: [10, 11, 12],
                                    15: [13, 14, 15], NJB: []}
                    else:
                        # absorb OP(ih=1, o=0) units at jb2..9; PV back-loaded
                        units = [(1, mt, c, [0], 1)
                                 for mt in range(8) for c in range(NC2)]
                        op_sched = {jb: units[2 * (jb - 2):2 * (jb - 2) + 2]
                                    for jb in range(2, 10)}
                        pv_sched = {9: [0], 10: [1, 2], 11: [3, 4],
                                    12: [5, 6], 13: [7, 8, 9],
                                    14: [10, 11, 12], 15: [13, 14, 15],
                                    NJB: []}

                    ess_all = {}
                    for jb in range(NJB):
                        pst = emit_S(jb)
                        ess_all[jb] = emit_exp(pst)
                        for u in op_sched.get(jb, []):
                            op_unit(*u)
                        for pv_jb in pv_sched.get(jb, []):
                            emit_PV(pv_jb, ess_all[pv_jb])
                    for pv_jb in pv_sched.get(NJB, []):
                        emit_PV(pv_jb, ess_all[pv_jb])

                    normalize(ih, pair, heads, pso, direct=(wi >= 1))

                # tail: OP(ih=1, o=1) units, c-major so the c0 normalize
                # outputs unblock the first half; evacs alternate ACT/DVE
                for c in range(NC2):
                    for mt in range(8):
                        op_unit(1, mt, c, [1], 0,
                                evac=("s" if mt % 2 else "v"))

    nc.compile()
    return nc


_NC_CACHE = None


def _get_nc():
    global _NC_CACHE
    if _NC_CACHE is None:
        _NC_CACHE = build_nc()
    return _NC_CACHE


def kernel(x, Wq, Wk, Wv, Wo, bo, _trace=False):
    x = np.asarray(x, dtype=DT_NP)
    Wq = np.asarray(Wq, dtype=DT_NP)
    Wk = np.asarray(Wk, dtype=DT_NP)
    Wv = np.asarray(Wv, dtype=DT_NP)
    Wo = np.asarray(Wo, dtype=DT_NP)
    bo = np.asarray(bo, dtype=DT_NP)
    B = x.shape[0]

    nc = _get_nc()
    in_maps = []
    for core in range(8):
        b, hg = divmod(core, 4)
        rows = slice(hg * DSH, (hg + 1) * DSH)
        in_maps.append({
            "xt": np.ascontiguousarray(x[b].T).astype(BF16_NP),
            "wqt": np.ascontiguousarray(Wq[rows, :].T).astype(BF16_NP),
            "wkt": np.ascontiguousarray((Wk[rows, :] * SCALE).T).astype(BF16_NP),
            "wvt": np.ascontiguousarray(Wv[rows, :].T).astype(BF16_NP),
            "wot": np.ascontiguousarray(Wo[:, rows].T).astype(BF16_NP),
        })

    res = bass_utils.run_bass_kernel_spmd(
        nc, in_maps, core_ids=list(range(8)), trace=_trace)

    out = np.zeros((B, NTOK, D), dtype=DT_NP)
    for core in range(8):
        b = core // 4
        out[b] += res.results[core]["outt"].T.astype(DT_NP)
        out[b, IHW:NTOK, :] += res.results[core]["out0"].T.astype(DT_NP)
    out += bo
    if _trace:
        kernel.last_results = res
    return out


# revision 23
# speedup vs baseline: 1.0240x; 1.0240x over previous
"""Multi-head attention (B=2, N=2048, D=1024, H=16, hd=64) on 8 TRN2 NeuronCores.

Sharding: data-parallel over batch (2) x tensor-parallel over heads (4 groups
of 4 heads). Each core computes, for its (batch b, head group g), the partial
output over its 256 sharded head dims; the host sums the 4 head-group
partials per batch, adds bo.

Per-core device kernel (all matmuls bf16 with fp32 PSUM accumulation):
  Phase A: xT/weights streamed over 3 DMA queues; Q^T/K^T (zero-padded
  two-heads-per-tile layout) and V|ones projections.
  Phase B: 4 windows (i-half x head-pair).  Per jb: S^T matmuls -> ACT exp
  (PSUM f32 -> SBUF bf16) -> PV accumulation (V|ones stationary gives
  Z = sum_j exp in PSUM row 64).  The output projection for i-half 0 is
  absorbed into window 3's jb loop (PV deferred to the window's back half so
  the o-tag PSUM banks are free early); the o=0 half of i-half 1's output
  projection is absorbed into window 4 the same way (host sums the o-split
  partials), leaving only the o=1 half on the tail.  Normalization uses
  reciprocal_approx_fast + gpsimd partition_broadcast (attn library); the
  pair-1 windows multiply PSUM directly to shorten the critical chain.
"""
import sys

sys.path.insert(0, "/opt/trn_rl_repo")

import numpy as np
import ml_dtypes

import concourse.bass as bass
import concourse.tile as tile
from concourse import bacc, bass_utils, mybir, library_config

P = 128
NTOK = 2048          # sequence length
D = 1024             # model dim
HPC = 4              # heads per core
HD = 64              # head dim
DSH = HPC * HD       # 256: sharded head dims per core
CO = D // P          # 8 contraction chunks over c
NIH = 2              # i halves
IHW = NTOK // NIH    # 1024
NC2 = IHW // 512     # 512-chunks per half = 2
NJB = NTOK // P      # 16 j blocks
SCALE = HD ** -0.5

F32 = mybir.dt.float32
BF16 = mybir.dt.bfloat16
DT_NP = np.float32
BF16_NP = ml_dtypes.bfloat16


def build_nc():
    nc = bacc.Bacc("TRN2", target_bir_lowering=False, debug=False)

    xt_d = nc.dram_tensor("xt", [D, NTOK], BF16, kind="ExternalInput").ap()
    wqt_d = nc.dram_tensor("wqt", [D, DSH], BF16, kind="ExternalInput").ap()
    wkt_d = nc.dram_tensor("wkt", [D, DSH], BF16, kind="ExternalInput").ap()
    wvt_d = nc.dram_tensor("wvt", [D, DSH], BF16, kind="ExternalInput").ap()
    wot_d = nc.dram_tensor("wot", [DSH, D], BF16, kind="ExternalInput").ap()
    outt_d = nc.dram_tensor("outt", [D, NTOK], BF16, kind="ExternalOutput").ap()
    out0_d = nc.dram_tensor("out0", [D, IHW], BF16, kind="ExternalOutput").ap()

    xt_t = xt_d.rearrange("(o p) i -> p o i", p=P)        # [128, 8, 2048]
    wq_t = wqt_d.rearrange("(o p) d -> p o d", p=P)       # [128, 8, 256]
    wk_t = wkt_d.rearrange("(o p) d -> p o d", p=P)
    wv_t = wvt_d.rearrange("(o p) d -> p o d", p=P)
    wo_t = wot_d.rearrange("(o p) e -> p o e", p=P)       # [128, 2, 1024]
    out_t = outt_d.rearrange("(m p) i -> p m i", p=P)     # [128, 8, 2048]
    out0_t = out0_d.rearrange("(m p) i -> p m i", p=P)    # [128, 8, 1024]

    with tile.TileContext(nc) as tc:
        with (
            tc.tile_pool(name="sbp", bufs=1) as sbp,           # persistent
            tc.tile_pool(name="ps", bufs=1, space="PSUM") as ps,
        ):
            # persistent activation tensors
            qt = sbp.tile([P, 2, NTOK], BF16, tag="qt")        # Q^T natural
            ktp = sbp.tile([P, HPC, NTOK], BF16, tag="ktp")    # K^T padded
            vaug = sbp.tile([P, NJB, HPC, 65], BF16, tag="vaug")  # V | ones
            ota = sbp.tile([P, 2, NTOK], BF16, tag="ota")      # O^T all heads
            wo = sbp.tile([P, 2, D], BF16, tag="wo")

            nc.gpsimd.load_library(library_config.attn)

            # zero rows of the padded K^T slots; ones column of vaug (DVE)
            nc.vector.memset(ktp[64:128, 0, :], 0.0)
            nc.vector.memset(ktp[0:64, 1, :], 0.0)
            nc.vector.memset(ktp[64:128, 2, :], 0.0)
            nc.vector.memset(ktp[0:64, 3, :], 0.0)
            nc.vector.memset(vaug[:, :, :, 64:65], 1.0)

            # psum tags: a0/a1 [128,1024] (2 banks each), o0..o3 (1 bank each)
            def psA(i):
                return ps.tile([P, IHW], F32, tag=f"a{i % 2}", name=f"psA{i % 2}")

            _ot_ctr = [0]

            def psO():
                i = _ot_ctr[0]
                _ot_ctr[0] += 1
                return ps.tile([P, 512], F32, tag=f"o{i % 4}", name=f"psO{i % 4}")

            # ------------- phase A: loads + projections (xt/w scoped) -------
            with tc.tile_pool(name="sbl", bufs=1) as sbl:
                wq = sbl.tile([P, CO, DSH], BF16, tag="wq")
                wk = sbl.tile([P, CO, DSH], BF16, tag="wk")
                wv = sbl.tile([P, CO, DSH], BF16, tag="wv")
                xt = [sbl.tile([P, NTOK], BF16, tag=f"xt{o}", name=f"xt{o}")
                      for o in range(CO)]
                # spread input loads over the two hardware DMA queues;
                # mt0 halves of Wq/Wk first so A1 can start immediately
                nc.scalar.dma_start(wq[:, :, 0:P], wq_t[:, :, 0:P])
                nc.sync.dma_start(wk[:, :, 0:P], wk_t[:, :, 0:P])
                for o in range(CO):
                    eng = nc.sync if o % 2 == 0 else nc.scalar
                    eng.dma_start(xt[o][:], xt_t[:, o, :])
                nc.scalar.dma_start(wq[:, :, P:DSH], wq_t[:, :, P:DSH])
                nc.sync.dma_start(wk[:, :, P:DSH], wk_t[:, :, P:DSH])
                nc.scalar.dma_start(wv[:], wv_t)
                nc.sync.dma_start(wo[:], wo_t)

                # A1: Q(mt0) + K(mt0), o-streamed so the PE starts on the
                # first xT chunk.  Q groups [128,1024] on a-tags; K groups
                # [128,512] per (ih,c) on o-tags.
                ppq = {ih: psA(ih) for ih in range(NIH)}
                ppk = {(ih, c): psO() for ih in range(NIH) for c in range(NC2)}
                for o in range(CO):
                    for ih in range(NIH):
                        for c in range(NC2):
                            nc.tensor.matmul(
                                ppq[ih][:, c * 512:(c + 1) * 512],
                                wq[:, o, 0:P],
                                xt[o][:, ih * IHW + c * 512:
                                      ih * IHW + (c + 1) * 512],
                                start=(o == 0), stop=(o == CO - 1),
                            )
                            nc.tensor.matmul(
                                ppk[(ih, c)][:],
                                wk[:, o, 0:P],
                                xt[o][:, ih * IHW + c * 512:
                                      ih * IHW + (c + 1) * 512],
                                start=(o == 0), stop=(o == CO - 1),
                            )
                for ih in range(NIH):
                    sl = slice(ih * IHW, (ih + 1) * IHW)
                    nc.scalar.copy(qt[:, 0, sl], ppq[ih][:, :])
                    for c in range(NC2):
                        slc = slice(ih * IHW + c * 512, ih * IHW + (c + 1) * 512)
                        nc.scalar.copy(ktp[0:64, 0, slc], ppk[(ih, c)][0:64, :])
                        nc.scalar.copy(ktp[64:128, 1, slc], ppk[(ih, c)][64:128, :])

                # A2: Q(mt1) / K(mt1) (all xT resident by now), then V
                for w_sb, is_q in ((wq, True), (wk, False)):
                    for ih in range(NIH):
                        pp = psA(ih)
                        for c in range(NC2):
                            for o in range(CO):
                                nc.tensor.matmul(
                                    pp[:, c * 512:(c + 1) * 512],
                                    w_sb[:, o, P:2 * P],
                                    xt[o][:, ih * IHW + c * 512:
                                          ih * IHW + (c + 1) * 512],
                                    start=(o == 0), stop=(o == CO - 1),
                                )
                        sl = slice(ih * IHW, (ih + 1) * IHW)
                        if is_q:
                            nc.scalar.copy(qt[:, 1, sl], pp[:, :])
                        else:
                            nc.scalar.copy(ktp[0:64, 2, sl], pp[0:64, :])
                            nc.scalar.copy(ktp[64:128, 3, sl], pp[64:128, :])

                # V projection
                for it in range(NJB):
                    pv = psO()
                    for o in range(CO):
                        nc.tensor.matmul(
                            pv[:, 0:DSH],
                            xt[o][:, it * P:(it + 1) * P],
                            wv[:, o, :],
                            start=(o == 0), stop=(o == CO - 1),
                        )
                    nc.scalar.copy(
                        vaug[:, it, :, 0:64],
                        pv[:, 0:DSH].rearrange("p (h d) -> p h d", d=HD),
                    )

            # ------------- phase B: attention + output projection -----------
            with tc.tile_pool(name="sbw", bufs=1) as sbw:
                ES_BUFS = 24

                def op_unit(ih, mt, c, o_list, dest, evac="v"):
                    """One output-projection unit: psum [128,512], matmuls
                    over o_list, evacuate bf16, DMA out."""
                    pe = psO()
                    n_o = len(o_list)
                    for i, o in enumerate(o_list):
                        nc.tensor.matmul(
                            pe[:],
                            wo[:, o, mt * P:(mt + 1) * P],
                            ota[:, o, ih * IHW + c * 512:
                                ih * IHW + (c + 1) * 512],
                            start=(i == 0), stop=(i == n_o - 1),
                        )
                    stg = sbw.tile([P, 512], BF16, tag="stg", bufs=4)
                    if evac == "v":
                        nc.vector.tensor_copy(stg[:], pe[:])
                    else:
                        nc.scalar.copy(stg[:], pe[:])
                    if dest == 0:
                        eng = nc.sync if mt % 2 == 0 else nc.scalar
                        eng.dma_start(
                            out_t[:, mt, ih * IHW + c * 512:
                                  ih * IHW + (c + 1) * 512], stg[:])
                    else:
                        nc.scalar.dma_start(
                            out0_t[:, mt, c * 512:(c + 1) * 512], stg[:])

                def normalize(ih, pair, heads, pso, direct):
                    """Scale O^T rows by 1/Z.  direct=True multiplies PSUM
                    in place (shorter chain, holds the o-tag banks); False
                    evacuates first (frees banks for the next window's PV).
                    The chain is spread across ACT (z-copies), DVE (recip,
                    mults) and Pool (broadcasts), c-major so the c0 outputs
                    finish first for downstream c-major consumers."""
                    ot, rb, rts = {}, {}, {}
                    for k, h in enumerate(heads):
                        zt = sbw.tile([1, IHW], F32, tag="zt", bufs=2)
                        for c in range(NC2):
                            nc.vector.tensor_copy(zt[:, c * 512:(c + 1) * 512],
                                                  pso[(h, c)][64:65, :])
                        rts[h] = sbw.tile([1, IHW], F32, tag="rt", bufs=2,
                                          name="rt")
                        nc.vector.reciprocal_approx_fast(out=rts[h][:],
                                                         in_=zt[:])
                        rb[h] = sbw.tile([64, IHW], F32, tag="rb", bufs=2,
                                         name="rb")
                        if not direct:
                            ot[h] = sbw.tile([64, IHW], F32, tag="otmp",
                                             bufs=2, name="otmp")
                            for c in range(NC2):
                                nc.vector.tensor_copy(
                                    ot[h][:, c * 512:(c + 1) * 512],
                                    pso[(h, c)][0:64, :])
                    for c in range(NC2):
                        for k, h in enumerate(heads):
                            nc.gpsimd.partition_broadcast(
                                rb[h][:, c * 512:(c + 1) * 512],
                                rts[h][:, c * 512:(c + 1) * 512])
                    for c in range(NC2):
                        for k, h in enumerate(heads):
                            row = slice((h % 2) * 64, (h % 2) * 64 + 64)
                            src = (pso[(h, c)][0:64, :] if direct
                                   else ot[h][:, c * 512:(c + 1) * 512])
                            nc.vector.tensor_mul(
                                ota[row, h // 2, ih * IHW + c * 512:
                                    ih * IHW + (c + 1) * 512],
                                src,
                                rb[h][:, c * 512:(c + 1) * 512],
                            )

                for wi, (ih, pair) in enumerate([(0, 0), (0, 1), (1, 0), (1, 1)]):
                    heads = (2 * pair, 2 * pair + 1)
                    pso = {}

                    def get_pso(h, c):
                        if (h, c) not in pso:
                            pso[(h, c)] = psO()
                        return pso[(h, c)]

                    def emit_S(jb):
                        pst = {}
                        for k, h in enumerate(heads):
                            pa = psA(k)
                            pst[h] = pa
                            for c in range(NC2):
                                nc.tensor.matmul(
                                    pa[:, c * 512:(c + 1) * 512],
                                    ktp[:, h, jb * P:(jb + 1) * P],
                                    qt[:, h // 2, ih * IHW + c * 512:
                                       ih * IHW + (c + 1) * 512],
                                    start=True, stop=True,
                                )
                        return pst

                    def emit_exp(pst):
                        ess = {}
                        for k, h in enumerate(heads):
                            es = sbw.tile([P, IHW], BF16, tag="es",
                                          bufs=ES_BUFS)
                            nc.scalar.activation(
                                es[:], pst[h][:],
                                mybir.ActivationFunctionType.Exp,
                            )
                            ess[h] = es
                        return ess

                    def emit_PV(jb, ess):
                        for k, h in enumerate(heads):
                            for c in range(NC2):
                                nc.tensor.matmul(
                                    get_pso(h, c)[0:65, :],
                                    vaug[:, jb, h, 0:65],
                                    ess[h][:, c * 512:(c + 1) * 512],
                                    start=(jb == 0), stop=(jb == NJB - 1),
                                )

                    # per-jb schedules: (op_units_emitted_this_jb, pvs)
                    if wi < 2:
                        op_sched = {}
                        pv_sched = {jb: [jb - 1] for jb in range(1, NJB)}
                        pv_sched[NJB] = [NJB - 1]
                    elif wi == 2:
                        # absorb OP(ih=0) full units at jb2..9 (after the
                        # previous window's normalize chain); PV deferred
                        units = [(0, mt, c, [0, 1], 0)
                                 for mt in range(8) for c in range(NC2)]
                        op_sched = {jb: units[2 * (jb - 2):2 * (jb - 2) + 2]
                                    for jb in range(2, 10)}
                        pv_sched = {10: [0, 1], 11: [2, 3], 12: [4, 5, 6],
                                    13: [7, 8, 9], 14: [10, 11, 12],
                                    15: [13, 14, 15], NJB: []}
                    else:
                        # absorb OP(ih=1, o=0) units at jb2..9; PV back-loaded
                        units = [(1, mt, c, [0], 1)
                                 for mt in range(8) for c in range(NC2)]
                        op_sched = {jb: units[2 * (jb - 2):2 * (jb - 2) + 2]
                                    for jb in range(2, 10)}
                        pv_sched = {9: [0], 10: [1, 2], 11: [3, 4],
                                    12: [5, 6], 13: [7, 8, 9],
                                    14: [10, 11, 12], 15: [13, 14, 15],
                                    NJB: []}

                    ess_all = {}
                    for jb in range(NJB):
                        pst = emit_S(jb)
                        ess_all[jb] = emit_exp(pst)
                        for u in op_sched.get(jb, []):
                            op_unit(*u)
                        for pv_jb in pv_sched.get(jb, []):
                            emit_PV(pv_jb, ess_all[pv_jb])
                    for pv_jb in pv_sched.get(NJB, []):
                        emit_PV(pv_jb, ess_all[pv_jb])

                    normalize(ih, pair, heads, pso, direct=(pair == 1))

                # tail: OP(ih=1, o=1) units, c-major so the c0 normalize
                # outputs unblock the first half; evacs alternate ACT/DVE
                for c in range(NC2):
                    for mt in range(8):
                        op_unit(1, mt, c, [1], 0,
                                evac=("s" if mt % 2 else "v"))

    nc.compile()
    return nc


_NC_CACHE = None


def _get_nc():
    global _NC_CACHE
    if _NC_CACHE is None:
        _NC_CACHE = build_nc()
    return _NC_CACHE


def kernel(x, Wq, Wk, Wv, Wo, bo, _trace=False):
    x = np.asarray(x, dtype=DT_NP)
    Wq = np.asarray(Wq, dtype=DT_NP)
    Wk = np.asarray(Wk, dtype=DT_NP)
    Wv = np.asarray(Wv, dtype=DT_NP)
    Wo = np.asarray(Wo, dtype=DT_NP)
    bo = np.asarray(bo, dtype=DT_NP)
    B = x.shape[0]

    nc = _get_nc()
    in_maps = []
    for core in range(8):
        b, hg = divmod(core, 4)
        rows = slice(hg * DSH, (hg + 1) * DSH)
        in_maps.append({
            "xt": np.ascontiguousarray(x[b].T).astype(BF16_NP),
            "wqt": np.ascontiguousarray(Wq[rows, :].T).astype(BF16_NP),
            "wkt": np.ascontiguousarray((Wk[rows, :] * SCALE).T).astype(BF16_NP),
            "wvt": np.ascontiguousarray(Wv[rows, :].T).astype(BF16_NP),
            "wot": np.ascontiguousarray(Wo[:, rows].T).astype(BF16_NP),
        })

    res = bass_utils.run_bass_kernel_spmd(
        nc, in_maps, core_ids=list(range(8)), trace=_trace)

    out = np.zeros((B, NTOK, D), dtype=DT_NP)
    for core in range(8):
        b = core // 4
        out[b] += res.results[core]["outt"].T.astype(DT_NP)
        out[b, IHW:NTOK, :] += res.results[core]["out0"].T.astype(DT_NP)
    out += bo
    if _trace:
        kernel.last_results = res
    return out


# revision 24
# speedup vs baseline: 1.0297x; 1.0056x over previous
"""Multi-head attention (B=2, N=2048, D=1024, H=16, hd=64) on 8 TRN2 NeuronCores.

Sharding: data-parallel over batch (2) x tensor-parallel over heads (4 groups
of 4 heads). Each core computes, for its (batch b, head group g), the partial
output over its 256 sharded head dims; the host sums the 4 head-group
partials per batch, adds bo.

Per-core device kernel (all matmuls bf16 with fp32 PSUM accumulation):
  Phase A: xT/weights streamed over both HW DMA queues (mt0 weight halves
  first); Q^T/K^T (zero-padded two-heads-per-tile layout) and V|ones
  projections, with the mt0 Q/K groups accumulating per xT chunk as it lands.
  Phase B: 4 windows (i-half x head-pair).  Per jb: S^T matmuls -> ACT exp
  (PSUM f32 -> SBUF bf16) -> PV accumulation (V|ones stationary gives
  Z = sum_j exp in PSUM row 64).  The output projection for i-half 0 is
  absorbed into window 3's jb loop (PV deferred to the window's back half so
  the o-tag PSUM banks are free early); the o=0 half of i-half 1's output
  projection is absorbed into window 4 the same way (host sums the o-split
  partials), leaving only the o=1 half on the tail.  Normalization uses
  reciprocal_approx_fast + gpsimd partition_broadcast (attn library); the
  pair-1 windows multiply PSUM directly to shorten the critical chain.
"""
import sys

sys.path.insert(0, "/opt/trn_rl_repo")

import numpy as np
import ml_dtypes

import concourse.bass as bass
import concourse.tile as tile
from concourse import bacc, bass_utils, mybir, library_config

P = 128
NTOK = 2048          # sequence length
D = 1024             # model dim
HPC = 4              # heads per core
HD = 64              # head dim
DSH = HPC * HD       # 256: sharded head dims per core
CO = D // P          # 8 contraction chunks over c
NIH = 2              # i halves
IHW = NTOK // NIH    # 1024
NC2 = IHW // 512     # 512-chunks per half = 2
NJB = NTOK // P      # 16 j blocks
SCALE = HD ** -0.5

F32 = mybir.dt.float32
BF16 = mybir.dt.bfloat16
DT_NP = np.float32
BF16_NP = ml_dtypes.bfloat16


def build_nc():
    nc = bacc.Bacc("TRN2", target_bir_lowering=False, debug=False)

    xt_d = nc.dram_tensor("xt", [D, NTOK], BF16, kind="ExternalInput").ap()
    wqt_d = nc.dram_tensor("wqt", [D, DSH], BF16, kind="ExternalInput").ap()
    wkt_d = nc.dram_tensor("wkt", [D, DSH], BF16, kind="ExternalInput").ap()
    wvt_d = nc.dram_tensor("wvt", [D, DSH], BF16, kind="ExternalInput").ap()
    wot_d = nc.dram_tensor("wot", [DSH, D], BF16, kind="ExternalInput").ap()
    outt_d = nc.dram_tensor("outt", [D, NTOK], BF16, kind="ExternalOutput").ap()
    out0_d = nc.dram_tensor("out0", [D, IHW], BF16, kind="ExternalOutput").ap()

    xt_t = xt_d.rearrange("(o p) i -> p o i", p=P)        # [128, 8, 2048]
    wq_t = wqt_d.rearrange("(o p) d -> p o d", p=P)       # [128, 8, 256]
    wk_t = wkt_d.rearrange("(o p) d -> p o d", p=P)
    wv_t = wvt_d.rearrange("(o p) d -> p o d", p=P)
    wo_t = wot_d.rearrange("(o p) e -> p o e", p=P)       # [128, 2, 1024]
    out_t = outt_d.rearrange("(m p) i -> p m i", p=P)     # [128, 8, 2048]
    out0_t = out0_d.rearrange("(m p) i -> p m i", p=P)    # [128, 8, 1024]

    with tile.TileContext(nc) as tc:
        with (
            tc.tile_pool(name="sbp", bufs=1) as sbp,           # persistent
            tc.tile_pool(name="ps", bufs=1, space="PSUM") as ps,
        ):
            # persistent activation tensors
            qt = sbp.tile([P, 2, NTOK], BF16, tag="qt")        # Q^T natural
            ktp = sbp.tile([P, HPC, NTOK], BF16, tag="ktp")    # K^T padded
            vaug = sbp.tile([P, NJB, HPC, 65], BF16, tag="vaug")  # V | ones
            ota = sbp.tile([P, 2, NTOK], BF16, tag="ota")      # O^T all heads
            wo = sbp.tile([P, 2, D], BF16, tag="wo")

            nc.gpsimd.load_library(library_config.attn)

            # zero rows of the padded K^T slots; ones column of vaug (DVE)
            nc.vector.memset(ktp[64:128, 0, :], 0.0)
            nc.vector.memset(ktp[0:64, 1, :], 0.0)
            nc.vector.memset(ktp[64:128, 2, :], 0.0)
            nc.vector.memset(ktp[0:64, 3, :], 0.0)
            nc.vector.memset(vaug[:, :, :, 64:65], 1.0)

            # psum tags: a0/a1 [128,1024] (2 banks each), o0..o3 (1 bank each)
            def psA(i):
                return ps.tile([P, IHW], F32, tag=f"a{i % 2}", name=f"psA{i % 2}")

            _ot_ctr = [0]

            def psO():
                i = _ot_ctr[0]
                _ot_ctr[0] += 1
                return ps.tile([P, 512], F32, tag=f"o{i % 4}", name=f"psO{i % 4}")

            # ------------- phase A: loads + projections (xt/w scoped) -------
            with tc.tile_pool(name="sbl", bufs=1) as sbl:
                wq = sbl.tile([P, CO, DSH], BF16, tag="wq")
                wk = sbl.tile([P, CO, DSH], BF16, tag="wk")
                wv = sbl.tile([P, CO, DSH], BF16, tag="wv")
                xt = [sbl.tile([P, NTOK], BF16, tag=f"xt{o}", name=f"xt{o}")
                      for o in range(CO)]
                # spread input loads over the two hardware DMA queues;
                # mt0 halves of Wq/Wk first so A1 can start immediately
                nc.scalar.dma_start(wq[:, :, 0:P], wq_t[:, :, 0:P])
                nc.sync.dma_start(wk[:, :, 0:P], wk_t[:, :, 0:P])
                for o in range(CO):
                    eng = nc.sync if o % 2 == 0 else nc.scalar
                    eng.dma_start(xt[o][:], xt_t[:, o, :])
                nc.scalar.dma_start(wq[:, :, P:DSH], wq_t[:, :, P:DSH])
                nc.sync.dma_start(wk[:, :, P:DSH], wk_t[:, :, P:DSH])
                nc.scalar.dma_start(wv[:], wv_t)
                nc.sync.dma_start(wo[:], wo_t)

                # A1: Q(mt0) + K(mt0), o-streamed so the PE starts on the
                # first xT chunk.  Q groups [128,1024] on a-tags; K groups
                # [128,512] per (ih,c) on o-tags.
                ppq = {ih: psA(ih) for ih in range(NIH)}
                ppk = {(ih, c): psO() for ih in range(NIH) for c in range(NC2)}
                for o in range(CO):
                    for ih in range(NIH):
                        for c in range(NC2):
                            nc.tensor.matmul(
                                ppq[ih][:, c * 512:(c + 1) * 512],
                                wq[:, o, 0:P],
                                xt[o][:, ih * IHW + c * 512:
                                      ih * IHW + (c + 1) * 512],
                                start=(o == 0), stop=(o == CO - 1),
                            )
                            nc.tensor.matmul(
                                ppk[(ih, c)][:],
                                wk[:, o, 0:P],
                                xt[o][:, ih * IHW + c * 512:
                                      ih * IHW + (c + 1) * 512],
                                start=(o == 0), stop=(o == CO - 1),
                            )
                for ih in range(NIH):
                    sl = slice(ih * IHW, (ih + 1) * IHW)
                    nc.scalar.copy(qt[:, 0, sl], ppq[ih][:, :])
                    for c in range(NC2):
                        slc = slice(ih * IHW + c * 512, ih * IHW + (c + 1) * 512)
                        nc.scalar.copy(ktp[0:64, 0, slc], ppk[(ih, c)][0:64, :])
                        nc.scalar.copy(ktp[64:128, 1, slc], ppk[(ih, c)][64:128, :])

                # A2: Q(mt1) / K(mt1) (all xT resident by now), then V
                for w_sb, is_q in ((wq, True), (wk, False)):
                    for ih in range(NIH):
                        pp = psA(ih)
                        for c in range(NC2):
                            for o in range(CO):
                                nc.tensor.matmul(
                                    pp[:, c * 512:(c + 1) * 512],
                                    w_sb[:, o, P:2 * P],
                                    xt[o][:, ih * IHW + c * 512:
                                          ih * IHW + (c + 1) * 512],
                                    start=(o == 0), stop=(o == CO - 1),
                                )
                        sl = slice(ih * IHW, (ih + 1) * IHW)
                        if is_q:
                            nc.scalar.copy(qt[:, 1, sl], pp[:, :])
                        else:
                            nc.scalar.copy(ktp[0:64, 2, sl], pp[0:64, :])
                            nc.scalar.copy(ktp[64:128, 3, sl], pp[64:128, :])

                # V projection
                for it in range(NJB):
                    pv = psO()
                    for o in range(CO):
                        nc.tensor.matmul(
                            pv[:, 0:DSH],
                            xt[o][:, it * P:(it + 1) * P],
                            wv[:, o, :],
                            start=(o == 0), stop=(o == CO - 1),
                        )
                    nc.scalar.copy(
                        vaug[:, it, :, 0:64],
                        pv[:, 0:DSH].rearrange("p (h d) -> p h d", d=HD),
                    )

            # ------------- phase B: attention + output projection -----------
            with tc.tile_pool(name="sbw", bufs=1) as sbw:
                ES_BUFS = 24

                def op_unit(ih, mt, c, o_list, dest, evac="v"):
                    """One output-projection unit: psum [128,512], matmuls
                    over o_list, evacuate bf16, DMA out."""
                    pe = psO()
                    n_o = len(o_list)
                    for i, o in enumerate(o_list):
                        nc.tensor.matmul(
                            pe[:],
                            wo[:, o, mt * P:(mt + 1) * P],
                            ota[:, o, ih * IHW + c * 512:
                                ih * IHW + (c + 1) * 512],
                            start=(i == 0), stop=(i == n_o - 1),
                        )
                    stg = sbw.tile([P, 512], BF16, tag="stg", bufs=4)
                    if evac == "v":
                        nc.vector.tensor_copy(stg[:], pe[:])
                    else:
                        nc.scalar.copy(stg[:], pe[:])
                    if dest == 0:
                        eng = nc.sync if mt % 2 == 0 else nc.scalar
                        eng.dma_start(
                            out_t[:, mt, ih * IHW + c * 512:
                                  ih * IHW + (c + 1) * 512], stg[:])
                    else:
                        nc.scalar.dma_start(
                            out0_t[:, mt, c * 512:(c + 1) * 512], stg[:])

                def normalize(ih, pair, heads, pso, direct):
                    """Scale O^T rows by 1/Z.  direct=True multiplies PSUM
                    in place (shorter chain, holds the o-tag banks); False
                    evacuates first (frees banks for the next window's PV).
                    The chain is spread across ACT (z-copies), DVE (recip,
                    mults) and Pool (broadcasts), c-major so the c0 outputs
                    finish first for downstream c-major consumers."""
                    ot, rb, rts = {}, {}, {}
                    for k, h in enumerate(heads):
                        zt = sbw.tile([1, IHW], F32, tag="zt", bufs=2)
                        for c in range(NC2):
                            nc.vector.tensor_copy(zt[:, c * 512:(c + 1) * 512],
                                                  pso[(h, c)][64:65, :])
                        rts[h] = sbw.tile([1, IHW], F32, tag="rt", bufs=2,
                                          name="rt")
                        nc.vector.reciprocal_approx_fast(out=rts[h][:],
                                                         in_=zt[:])
                        rb[h] = sbw.tile([64, IHW], F32, tag="rb", bufs=2,
                                         name="rb")
                        if not direct:
                            ot[h] = sbw.tile([64, IHW], F32, tag="otmp",
                                             bufs=2, name="otmp")
                            for c in range(NC2):
                                nc.vector.tensor_copy(
                                    ot[h][:, c * 512:(c + 1) * 512],
                                    pso[(h, c)][0:64, :])
                    for c in range(NC2):
                        for k, h in enumerate(heads):
                            nc.gpsimd.partition_broadcast(
                                rb[h][:, c * 512:(c + 1) * 512],
                                rts[h][:, c * 512:(c + 1) * 512])
                    for c in range(NC2):
                        for k, h in enumerate(heads):
                            row = slice((h % 2) * 64, (h % 2) * 64 + 64)
                            src = (pso[(h, c)][0:64, :] if direct
                                   else ot[h][:, c * 512:(c + 1) * 512])
                            nc.vector.tensor_mul(
                                ota[row, h // 2, ih * IHW + c * 512:
                                    ih * IHW + (c + 1) * 512],
                                src,
                                rb[h][:, c * 512:(c + 1) * 512],
                            )

                for wi, (ih, pair) in enumerate([(0, 0), (0, 1), (1, 0), (1, 1)]):
                    heads = (2 * pair, 2 * pair + 1)
                    pso = {}

                    def get_pso(h, c):
                        if (h, c) not in pso:
                            pso[(h, c)] = psO()
                        return pso[(h, c)]

                    def emit_S(jb):
                        pst = {}
                        for k, h in enumerate(heads):
                            pa = psA(k)
                            pst[h] = pa
                            for c in range(NC2):
                                nc.tensor.matmul(
                                    pa[:, c * 512:(c + 1) * 512],
                                    ktp[:, h, jb * P:(jb + 1) * P],
                                    qt[:, h // 2, ih * IHW + c * 512:
                                       ih * IHW + (c + 1) * 512],
                                    start=True, stop=True,
                                )
                        return pst

                    def emit_exp(pst):
                        ess = {}
                        for k, h in enumerate(heads):
                            es = sbw.tile([P, IHW], BF16, tag="es",
                                          bufs=ES_BUFS)
                            nc.scalar.activation(
                                es[:], pst[h][:],
                                mybir.ActivationFunctionType.Exp,
                            )
                            ess[h] = es
                        return ess

                    def emit_PV(jb, ess):
                        for k, h in enumerate(heads):
                            for c in range(NC2):
                                nc.tensor.matmul(
                                    get_pso(h, c)[0:65, :],
                                    vaug[:, jb, h, 0:65],
                                    ess[h][:, c * 512:(c + 1) * 512],
                                    start=(jb == 0), stop=(jb == NJB - 1),
                                )

                    # per-jb schedules: (op_units_emitted_this_jb, pvs)
                    if wi < 2:
                        op_sched = {}
                        pv_sched = {jb: [jb - 1] for jb in range(1, NJB)}
                        pv_sched[NJB] = [NJB - 1]
                    elif wi == 2:
                        # absorb OP(ih=0) full units at jb2..9 (after the
                        # previous window's normalize chain); PV deferred
                        units = [(0, mt, c, [0, 1], 0)
                                 for mt in range(8) for c in range(NC2)]
                        op_sched = {jb: units[2 * (jb - 2):2 * (jb - 2) + 2]
                                    for jb in range(2, 10)}
                        pv_sched = {10: [0, 1], 11: [2, 3], 12: [4, 5, 6],
                                    13: [7, 8, 9], 14: [10, 11, 12],
                                    15: [13, 14, 15], NJB: []}
                    else:
                        # absorb OP(ih=1, o=0) units at jb2..9; PV back-loaded
                        units = [(1, mt, c, [0], 1)
                                 for mt in range(8) for c in range(NC2)]
                        op_sched = {jb: units[2 * (jb - 2):2 * (jb - 2) + 2]
                                    for jb in range(2, 10)}
                        pv_sched = {9: [0], 10: [1, 2], 11: [3, 4],
                                    12: [5, 6], 13: [7, 8, 9],
                                    14: [10, 11, 12], 15: [13, 14, 15],
                                    NJB: []}

                    ess_all = {}
                    for jb in range(NJB):
                        pst = emit_S(jb)
                        ess_all[jb] = emit_exp(pst)
                        for u in op_sched.get(jb, []):
                            op_unit(*u)
                        for pv_jb in pv_sched.get(jb, []):
                            emit_PV(pv_jb, ess_all[pv_jb])
                    for pv_jb in pv_sched.get(NJB, []):
                        emit_PV(pv_jb, ess_all[pv_jb])

                    normalize(ih, pair, heads, pso, direct=(pair == 1))

                # tail: OP(ih=1, o=1) units, c-major so the c0 normalize
                # outputs unblock the first half; evacs alternate ACT/DVE
                for c in range(NC2):
                    for mt in range(8):
                        op_unit(1, mt, c, [1], 0,
                                evac=("s" if mt % 2 else "v"))

    nc.compile()
    return nc


_NC_CACHE = None


def _get_nc():
    global _NC_CACHE
    if _NC_CACHE is None:
        _NC_CACHE = build_nc()
    return _NC_CACHE


def kernel(x, Wq, Wk, Wv, Wo, bo, _trace=False):
    x = np.asarray(x, dtype=DT_NP)
    Wq = np.asarray(Wq, dtype=DT_NP)
    Wk = np.asarray(Wk, dtype=DT_NP)
    Wv = np.asarray(Wv, dtype=DT_NP)
    Wo = np.asarray(Wo, dtype=DT_NP)
    bo = np.asarray(bo, dtype=DT_NP)
    B = x.shape[0]

    nc = _get_nc()
    in_maps = []
    for core in range(8):
        b, hg = divmod(core, 4)
        rows = slice(hg * DSH, (hg + 1) * DSH)
        in_maps.append({
            "xt": np.ascontiguousarray(x[b].T).astype(BF16_NP),
            "wqt": np.ascontiguousarray(Wq[rows, :].T).astype(BF16_NP),
            "wkt": np.ascontiguousarray((Wk[rows, :] * SCALE).T).astype(BF16_NP),
            "wvt": np.ascontiguousarray(Wv[rows, :].T).astype(BF16_NP),
            "wot": np.ascontiguousarray(Wo[:, rows].T).astype(BF16_NP),
        })

    res = bass_utils.run_bass_kernel_spmd(
        nc, in_maps, core_ids=list(range(8)), trace=_trace)

    out = np.zeros((B, NTOK, D), dtype=DT_NP)
    for core in range(8):
        b = core // 4
        out[b] += res.results[core]["outt"].T.astype(DT_NP)
        out[b, IHW:NTOK, :] += res.results[core]["out0"].T.astype(DT_NP)
    out += bo
    if _trace:
        kernel.last_results = res
    return out


# revision 31
# speedup vs baseline: 1.0438x; 1.0137x over previous
"""Multi-head attention (B=2, N=2048, D=1024, H=16, hd=64) on 8 TRN2 NeuronCores.

Sharding: data-parallel over batch (2) x tensor-parallel over heads (4 groups
of 4 heads). Each core computes, for its (batch b, head group g), the partial
output over its 256 sharded head dims; the host sums the 4 head-group
partials per batch, adds bo.

Per-core device kernel (all matmuls bf16 with fp32 PSUM accumulation):
  Phase A: xT/weights streamed over both HW DMA queues (mt0 weight halves
  first); Q^T/K^T (zero-padded two-heads-per-tile layout) and V|ones
  projections, with the mt0 Q/K groups accumulating per xT chunk as it lands.
  Phase B: 4 windows (i-half x head-pair).  Per jb: S^T matmuls -> ACT exp
  (PSUM f32 -> SBUF bf16) -> PV accumulation (V|ones stationary gives
  Z = sum_j exp in PSUM row 64).  The output projection for i-half 0 is
  absorbed into window 3's jb loop (PV deferred to the window's back half so
  the o-tag PSUM banks are free early); the o=0 half of i-half 1's output
  projection is absorbed into window 4 the same way (host sums the o-split
  partials), leaving only the o=1 half on the tail.  Normalization uses
  reciprocal_approx_fast + gpsimd partition_broadcast (attn library); the
  pair-1 windows multiply PSUM directly to shorten the critical chain.
"""
import sys

sys.path.insert(0, "/opt/trn_rl_repo")

import numpy as np
import ml_dtypes

import concourse.bass as bass
import concourse.tile as tile
from concourse import bacc, bass_utils, mybir, library_config

P = 128
NTOK = 2048          # sequence length
D = 1024             # model dim
HPC = 4              # heads per core
HD = 64              # head dim
DSH = HPC * HD       # 256: sharded head dims per core
CO = D // P          # 8 contraction chunks over c
NIH = 2              # i halves
IHW = NTOK // NIH    # 1024
NC2 = IHW // 512     # 512-chunks per half = 2
NJB = NTOK // P      # 16 j blocks
SCALE = HD ** -0.5

F32 = mybir.dt.float32
BF16 = mybir.dt.bfloat16
DT_NP = np.float32
BF16_NP = ml_dtypes.bfloat16


def build_nc():
    nc = bacc.Bacc("TRN2", target_bir_lowering=False, debug=False)

    xt_d = nc.dram_tensor("xt", [D, NTOK], BF16, kind="ExternalInput").ap()
    wqt_d = nc.dram_tensor("wqt", [D, DSH], BF16, kind="ExternalInput").ap()
    wkt_d = nc.dram_tensor("wkt", [D, DSH], BF16, kind="ExternalInput").ap()
    wvt_d = nc.dram_tensor("wvt", [D, DSH], BF16, kind="ExternalInput").ap()
    wot_d = nc.dram_tensor("wot", [DSH, D], BF16, kind="ExternalInput").ap()
    outt_d = nc.dram_tensor("outt", [D, NTOK], BF16, kind="ExternalOutput").ap()
    out0_d = nc.dram_tensor("out0", [D, IHW], BF16, kind="ExternalOutput").ap()

    xt_t = xt_d.rearrange("(o p) i -> p o i", p=P)        # [128, 8, 2048]
    wq_t = wqt_d.rearrange("(o p) d -> p o d", p=P)       # [128, 8, 256]
    wk_t = wkt_d.rearrange("(o p) d -> p o d", p=P)
    wv_t = wvt_d.rearrange("(o p) d -> p o d", p=P)
    wo_t = wot_d.rearrange("(o p) e -> p o e", p=P)       # [128, 2, 1024]
    out_t = outt_d.rearrange("(m p) i -> p m i", p=P)     # [128, 8, 2048]
    out0_t = out0_d.rearrange("(m p) i -> p m i", p=P)    # [128, 8, 1024]

    with tile.TileContext(nc) as tc:
        with (
            tc.tile_pool(name="sbp", bufs=1) as sbp,           # persistent
            tc.tile_pool(name="ps", bufs=1, space="PSUM") as ps,
        ):
            # persistent activation tensors
            qt = sbp.tile([P, 2, NTOK], BF16, tag="qt")        # Q^T natural
            ktp = sbp.tile([P, HPC, NTOK], BF16, tag="ktp")    # K^T padded
            vaug = sbp.tile([P, NJB, HPC, 65], BF16, tag="vaug")  # V | ones
            ota = sbp.tile([P, 2, NTOK], BF16, tag="ota")      # O^T all heads
            wo = sbp.tile([P, 2, D], BF16, tag="wo")

            nc.gpsimd.load_library(library_config.attn)

            # zero rows of the padded K^T slots; ones column of vaug (DVE)
            nc.vector.memset(ktp[64:128, 0, :], 0.0)
            nc.vector.memset(ktp[0:64, 1, :], 0.0)
            nc.vector.memset(ktp[64:128, 2, :], 0.0)
            nc.vector.memset(ktp[0:64, 3, :], 0.0)
            nc.vector.memset(vaug[:, :, :, 64:65], 1.0)

            # psum tags: a0/a1 [128,1024] (2 banks each), o0..o3 (1 bank each)
            def psA(i):
                return ps.tile([P, IHW], F32, tag=f"a{i % 2}", name=f"psA{i % 2}")

            _ot_ctr = [0]

            def psO():
                i = _ot_ctr[0]
                _ot_ctr[0] += 1
                return ps.tile([P, 512], F32, tag=f"o{i % 4}", name=f"psO{i % 4}")

            # ------------- phase A: loads + projections (xt/w scoped) -------
            with tc.tile_pool(name="sbl", bufs=1) as sbl:
                wq = sbl.tile([P, CO, DSH], BF16, tag="wq")
                wk = sbl.tile([P, CO, DSH], BF16, tag="wk")
                wv = sbl.tile([P, CO, DSH], BF16, tag="wv")
                xt = [sbl.tile([P, NTOK], BF16, tag=f"xt{o}", name=f"xt{o}")
                      for o in range(CO)]
                # spread input loads over the two hardware DMA queues;
                # mt0 halves of Wq/Wk first so A1 can start immediately
                nc.scalar.dma_start(wq[:, :, 0:P], wq_t[:, :, 0:P])
                nc.sync.dma_start(wk[:, :, 0:P], wk_t[:, :, 0:P])
                for o in range(CO):
                    eng = nc.sync if o % 2 == 0 else nc.scalar
                    eng.dma_start(xt[o][:], xt_t[:, o, :])
                nc.scalar.dma_start(wq[:, :, P:DSH], wq_t[:, :, P:DSH])
                nc.sync.dma_start(wk[:, :, P:DSH], wk_t[:, :, P:DSH])
                nc.scalar.dma_start(wv[:], wv_t)
                nc.sync.dma_start(wo[:], wo_t)

                # A1: Q(mt0) + K(mt0), o-streamed so the PE starts on the
                # first xT chunk.  Q groups [128,1024] on a-tags; K groups
                # [128,512] per (ih,c) on o-tags.
                ppq = {ih: psA(ih) for ih in range(NIH)}
                ppk = {(ih, c): psO() for ih in range(NIH) for c in range(NC2)}
                for o in range(CO):
                    for ih in range(NIH):
                        for c in range(NC2):
                            nc.tensor.matmul(
                                ppq[ih][:, c * 512:(c + 1) * 512],
                                wq[:, o, 0:P],
                                xt[o][:, ih * IHW + c * 512:
                                      ih * IHW + (c + 1) * 512],
                                start=(o == 0), stop=(o == CO - 1),
                            )
                            nc.tensor.matmul(
                                ppk[(ih, c)][:],
                                wk[:, o, 0:P],
                                xt[o][:, ih * IHW + c * 512:
                                      ih * IHW + (c + 1) * 512],
                                start=(o == 0), stop=(o == CO - 1),
                            )
                for ih in range(NIH):
                    sl = slice(ih * IHW, (ih + 1) * IHW)
                    nc.scalar.copy(qt[:, 0, sl], ppq[ih][:, :])
                    for c in range(NC2):
                        slc = slice(ih * IHW + c * 512, ih * IHW + (c + 1) * 512)
                        nc.scalar.copy(ktp[0:64, 0, slc], ppk[(ih, c)][0:64, :])
                        nc.scalar.copy(ktp[64:128, 1, slc], ppk[(ih, c)][64:128, :])

                # A2: Q(mt1) / K(mt1) (all xT resident by now), then V
                for w_sb, is_q in ((wq, True), (wk, False)):
                    for ih in range(NIH):
                        pp = psA(ih)
                        for c in range(NC2):
                            for o in range(CO):
                                nc.tensor.matmul(
                                    pp[:, c * 512:(c + 1) * 512],
                                    w_sb[:, o, P:2 * P],
                                    xt[o][:, ih * IHW + c * 512:
                                          ih * IHW + (c + 1) * 512],
                                    start=(o == 0), stop=(o == CO - 1),
                                )
                        sl = slice(ih * IHW, (ih + 1) * IHW)
                        if is_q:
                            nc.scalar.copy(qt[:, 1, sl], pp[:, :])
                        else:
                            nc.scalar.copy(ktp[0:64, 2, sl], pp[0:64, :])
                            nc.scalar.copy(ktp[64:128, 3, sl], pp[64:128, :])

                # V projection
                for it in range(NJB):
                    pv = psO()
                    for o in range(CO):
                        nc.tensor.matmul(
                            pv[:, 0:DSH],
                            xt[o][:, it * P:(it + 1) * P],
                            wv[:, o, :],
                            start=(o == 0), stop=(o == CO - 1),
                        )
                    nc.scalar.copy(
                        vaug[:, it, :, 0:64],
                        pv[:, 0:DSH].rearrange("p (h d) -> p h d", d=HD),
                    )

            # ------------- phase B: attention + output projection -----------
            with tc.tile_pool(name="sbw", bufs=1) as sbw:
                ES_BUFS = 24

                def op_unit(ih, mt, c, o_list, dest, evac="v", pe=None):
                    """One output-projection unit: psum [128,512], matmuls
                    over o_list, evacuate bf16, DMA out."""
                    if pe is None:
                        pe = psO()
                    n_o = len(o_list)
                    for i, o in enumerate(o_list):
                        nc.tensor.matmul(
                            pe[:],
                            wo[:, o, mt * P:(mt + 1) * P],
                            ota[:, o, ih * IHW + c * 512:
                                ih * IHW + (c + 1) * 512],
                            start=(i == 0), stop=(i == n_o - 1),
                        )
                    stg = sbw.tile([P, 512], BF16, tag="stg", bufs=6)
                    if evac == "v":
                        nc.vector.tensor_copy(stg[:], pe[:])
                    else:
                        nc.scalar.copy(stg[:], pe[:])
                    if dest == 0:
                        eng = nc.sync if mt % 2 == 0 else nc.scalar
                        eng.dma_start(
                            out_t[:, mt, ih * IHW + c * 512:
                                  ih * IHW + (c + 1) * 512], stg[:])
                    else:
                        nc.scalar.dma_start(
                            out0_t[:, mt, c * 512:(c + 1) * 512], stg[:])

                def normalize(ih, pair, heads, pso, direct):
                    """Scale O^T rows by 1/Z.  direct=True multiplies PSUM
                    in place (shorter chain, holds the o-tag banks); False
                    evacuates first (frees banks for the next window's PV).
                    direct path runs fully c-major (z, recip, broadcast,
                    mult per c-half) so the c0 outputs unblock downstream
                    c-major consumers as early as possible."""
                    rb, rts, zts, ot = {}, {}, {}, {}
                    for k, h in enumerate(heads):
                        zts[h] = sbw.tile([1, IHW], F32, tag="zt", bufs=2,
                                          name="zt")
                        rts[h] = sbw.tile([1, IHW], F32, tag="rt", bufs=2,
                                          name="rt")
                        rb[h] = sbw.tile([64, IHW], F32, tag="rb", bufs=2,
                                         name="rb")
                    if direct:
                        for c in range(NC2):
                            cs = slice(c * 512, (c + 1) * 512)
                            for k, h in enumerate(heads):
                                nc.vector.tensor_copy(zts[h][:, cs],
                                                      pso[(h, c)][64:65, :])
                            for k, h in enumerate(heads):
                                nc.vector.reciprocal_approx_fast(
                                    out=rts[h][:, cs], in_=zts[h][:, cs])
                            for k, h in enumerate(heads):
                                nc.gpsimd.partition_broadcast(
                                    rb[h][:, cs], rts[h][:, cs])
                            for k, h in enumerate(heads):
                                row = slice((h % 2) * 64, (h % 2) * 64 + 64)
                                nc.vector.tensor_mul(
                                    ota[row, h // 2, ih * IHW + c * 512:
                                        ih * IHW + (c + 1) * 512],
                                    pso[(h, c)][0:64, :],
                                    rb[h][:, cs],
                                )
                        return
                    for k, h in enumerate(heads):
                        for c in range(NC2):
                            nc.vector.tensor_copy(
                                zts[h][:, c * 512:(c + 1) * 512],
                                pso[(h, c)][64:65, :])
                        nc.vector.reciprocal_approx_fast(out=rts[h][:],
                                                         in_=zts[h][:])
                        ot[h] = sbw.tile([64, IHW], F32, tag="otmp",
                                         bufs=2, name="otmp")
                        for c in range(NC2):
                            nc.vector.tensor_copy(
                                ot[h][:, c * 512:(c + 1) * 512],
                                pso[(h, c)][0:64, :])
                    for c in range(NC2):
                        for k, h in enumerate(heads):
                            nc.gpsimd.partition_broadcast(
                                rb[h][:, c * 512:(c + 1) * 512],
                                rts[h][:, c * 512:(c + 1) * 512])
                    for c in range(NC2):
                        for k, h in enumerate(heads):
                            row = slice((h % 2) * 64, (h % 2) * 64 + 64)
                            nc.vector.tensor_mul(
                                ota[row, h // 2, ih * IHW + c * 512:
                                    ih * IHW + (c + 1) * 512],
                                ot[h][:, c * 512:(c + 1) * 512],
                                rb[h][:, c * 512:(c + 1) * 512],
                            )

                for wi, (ih, pair) in enumerate([(0, 0), (0, 1), (1, 0), (1, 1)]):
                    heads = (2 * pair, 2 * pair + 1)
                    pso = {}

                    def get_pso(h, c):
                        if (h, c) not in pso:
                            pso[(h, c)] = psO()
                        return pso[(h, c)]

                    def emit_S(jb):
                        pst = {}
                        for k, h in enumerate(heads):
                            pa = psA(k)
                            pst[h] = pa
                            for c in range(NC2):
                                nc.tensor.matmul(
                                    pa[:, c * 512:(c + 1) * 512],
                                    ktp[:, h, jb * P:(jb + 1) * P],
                                    qt[:, h // 2, ih * IHW + c * 512:
                                       ih * IHW + (c + 1) * 512],
                                    start=True, stop=True,
                                )
                        return pst

                    def emit_exp(pst):
                        ess = {}
                        for k, h in enumerate(heads):
                            es = sbw.tile([P, IHW], BF16, tag="es",
                                          bufs=ES_BUFS)
                            nc.scalar.activation(
                                es[:], pst[h][:],
                                mybir.ActivationFunctionType.Exp,
                            )
                            ess[h] = es
                        return ess

                    def emit_PV(jb, ess):
                        for k, h in enumerate(heads):
                            for c in range(NC2):
                                nc.tensor.matmul(
                                    get_pso(h, c)[0:65, :],
                                    vaug[:, jb, h, 0:65],
                                    ess[h][:, c * 512:(c + 1) * 512],
                                    start=(jb == 0), stop=(jb == NJB - 1),
                                )

                    # per-jb schedules: (op_units_emitted_this_jb, pvs)
                    if wi < 2:
                        op_sched = {}
                        pv_sched = {jb: [jb - 1] for jb in range(1, NJB)}
                        pv_sched[NJB] = [NJB - 1]
                    elif wi == 2:
                        # absorb OP(ih=0) full units at jb2..9 (after the
                        # previous window's normalize chain); PV deferred
                        units = [(0, mt, c, [0, 1], 0)
                                 for c in range(NC2) for mt in range(8)]
                        op_sched = {jb: units[2 * (jb - 2):2 * (jb - 2) + 2]
                                    for jb in range(2, 10)}
                        pv_sched = {10: [0, 1], 11: [2, 3], 12: [4, 5, 6],
                                    13: [7, 8, 9], 14: [10, 11, 12],
                                    15: [13, 14, 15], NJB: []}
                    else:
                        # absorb OP(ih=1, o=0) units at jb2..9; PV back-loaded
                        units = [(1, mt, c, [0], 1)
                                 for c in range(NC2) for mt in range(8)]
                        op_sched = {jb: units[2 * (jb - 2):2 * (jb - 2) + 2]
                                    for jb in range(2, 10)}
                        pv_sched = {9: [0], 10: [1, 2], 11: [3, 4],
                                    12: [5, 6], 13: [7, 8, 9],
                                    14: [10, 11, 12], 15: [13, 14, 15],
                                    NJB: []}

                    ess_all = {}
                    for jb in range(NJB):
                        pst = emit_S(jb)
                        ess_all[jb] = emit_exp(pst)
                        for u in op_sched.get(jb, []):
                            op_unit(*u)
                        for pv_jb in pv_sched.get(jb, []):
                            emit_PV(pv_jb, ess_all[pv_jb])
                    for pv_jb in pv_sched.get(NJB, []):
                        emit_PV(pv_jb, ess_all[pv_jb])

                    normalize(ih, pair, heads, pso, direct=(wi >= 1))

                # tail: OP(ih=1, o=1) units, c-major so the c0 normalize
                # outputs unblock the first half; evacs alternate ACT/DVE;
                # psum rotates over 6 tags (the a-tags are free by now)
                tail_tags = ["o0", "o1", "o2", "o3", "a0", "a1"]
                for i, (c, mt) in enumerate(
                        [(c, mt) for c in range(NC2) for mt in range(8)]):
                    pe = ps.tile([P, 512], F32, tag=tail_tags[i % 6],
                                 name=f"psT{i % 6}")
                    op_unit(1, mt, c, [1], 0,
                            evac=("s" if mt % 2 else "v"), pe=pe)

    nc.compile()
    return nc


_NC_CACHE = None


def _get_nc():
    global _NC_CACHE
    if _NC_CACHE is None:
        _NC_CACHE = build_nc()
    return _NC_CACHE


def kernel(x, Wq, Wk, Wv, Wo, bo, _trace=False):
    x = np.asarray(x, dtype=DT_NP)
    Wq = np.asarray(Wq, dtype=DT_NP)
    Wk = np.asarray(Wk, dtype=DT_NP)
    Wv = np.asarray(Wv, dtype=DT_NP)
    Wo = np.asarray(Wo, dtype=DT_NP)
    bo = np.asarray(bo, dtype=DT_NP)
    B = x.shape[0]

    nc = _get_nc()
    in_maps = []
    for core in range(8):
        b, hg = divmod(core, 4)
        rows = slice(hg * DSH, (hg + 1) * DSH)
        in_maps.append({
            "xt": np.ascontiguousarray(x[b].T).astype(BF16_NP),
            "wqt": np.ascontiguousarray(Wq[rows, :].T).astype(BF16_NP),
            "wkt": np.ascontiguousarray((Wk[rows, :] * SCALE).T).astype(BF16_NP),
            "wvt": np.ascontiguousarray(Wv[rows, :].T).astype(BF16_NP),
            "wot": np.ascontiguousarray(Wo[:, rows].T).astype(BF16_NP),
        })

    res = bass_utils.run_bass_kernel_spmd(
        nc, in_maps, core_ids=list(range(8)), trace=_trace)

    out = np.zeros((B, NTOK, D), dtype=DT_NP)
    for core in range(8):
        b = core // 4
        out[b] += res.results[core]["outt"].T.astype(DT_NP)
        out[b, IHW:NTOK, :] += res.results[core]["out0"].T.astype(DT_NP)
    out += bo
    if _trace:
        kernel.last_results = res
    return out


# revision 36
# speedup vs baseline: 1.0556x; 1.0113x over previous
"""Multi-head attention (B=2, N=2048, D=1024, H=16, hd=64) on 8 TRN2 NeuronCores.

Sharding: data-parallel over batch (2) x tensor-parallel over heads (4 groups
of 4 heads). Each core computes, for its (batch b, head group g), the partial
output over its 256 sharded head dims; the host sums the 4 head-group
partials per batch, adds bo.

Per-core device kernel (all matmuls bf16 with fp32 PSUM accumulation):
  Phase A: xT/weights streamed over both HW DMA queues (mt0 weight halves
  first); Q^T/K^T (zero-padded two-heads-per-tile layout) and V|ones
  projections, with the mt0 Q/K groups accumulating per xT chunk as it lands.
  Phase B: 4 windows (i-half x head-pair).  Per jb: S^T matmuls -> ACT exp
  (PSUM f32 -> SBUF bf16) -> PV accumulation (V|ones stationary gives
  Z = sum_j exp in PSUM row 64).  The output projection for i-half 0 is
  absorbed into window 3's jb loop (PV deferred to the window's back half so
  the o-tag PSUM banks are free early); the o=0 half of i-half 1's output
  projection is absorbed into window 4 the same way (host sums the o-split
  partials), leaving only the o=1 half on the tail.  Normalization uses
  reciprocal_approx_fast + gpsimd partition_broadcast (attn library); the
  pair-1 windows multiply PSUM directly to shorten the critical chain.
"""
import sys

sys.path.insert(0, "/opt/trn_rl_repo")

import numpy as np
import ml_dtypes

import concourse.bass as bass
import concourse.tile as tile
from concourse import bacc, bass_utils, mybir, library_config

P = 128
NTOK = 2048          # sequence length
D = 1024             # model dim
HPC = 4              # heads per core
HD = 64              # head dim
DSH = HPC * HD       # 256: sharded head dims per core
CO = D // P          # 8 contraction chunks over c
NIH = 2              # i halves
IHW = NTOK // NIH    # 1024
NC2 = IHW // 512     # 512-chunks per half = 2
NJB = NTOK // P      # 16 j blocks
SCALE = HD ** -0.5

F32 = mybir.dt.float32
BF16 = mybir.dt.bfloat16
DT_NP = np.float32
BF16_NP = ml_dtypes.bfloat16


def build_nc():
    nc = bacc.Bacc("TRN2", target_bir_lowering=False, debug=False)

    xt_d = nc.dram_tensor("xt", [D, NTOK], BF16, kind="ExternalInput").ap()
    wqt_d = nc.dram_tensor("wqt", [D, DSH], BF16, kind="ExternalInput").ap()
    wkt_d = nc.dram_tensor("wkt", [D, DSH], BF16, kind="ExternalInput").ap()
    wvt_d = nc.dram_tensor("wvt", [D, DSH], BF16, kind="ExternalInput").ap()
    wot_d = nc.dram_tensor("wot", [DSH, D], BF16, kind="ExternalInput").ap()
    outt_d = nc.dram_tensor("outt", [D, NTOK], BF16, kind="ExternalOutput").ap()
    out0_d = nc.dram_tensor("out0", [D, IHW], BF16, kind="ExternalOutput").ap()

    xt_t = xt_d.rearrange("(o p) i -> p o i", p=P)        # [128, 8, 2048]
    wq_t = wqt_d.rearrange("(o p) d -> p o d", p=P)       # [128, 8, 256]
    wk_t = wkt_d.rearrange("(o p) d -> p o d", p=P)
    wv_t = wvt_d.rearrange("(o p) d -> p o d", p=P)
    wo_t = wot_d.rearrange("(o p) e -> p o e", p=P)       # [128, 2, 1024]
    out_t = outt_d.rearrange("(m p) i -> p m i", p=P)     # [128, 8, 2048]
    out0_t = out0_d.rearrange("(m p) i -> p m i", p=P)    # [128, 8, 1024]

    with tile.TileContext(nc) as tc:
        with (
            tc.tile_pool(name="sbp", bufs=1) as sbp,           # persistent
            tc.tile_pool(name="ps", bufs=1, space="PSUM") as ps,
        ):
            # persistent activation tensors
            qt = sbp.tile([P, 2, NTOK], BF16, tag="qt")        # Q^T natural
            ktp = sbp.tile([P, HPC, NTOK], BF16, tag="ktp")    # K^T padded
            vaug = sbp.tile([P, NJB, HPC, 65], BF16, tag="vaug")  # V | ones
            ota = sbp.tile([P, 2, NTOK], BF16, tag="ota")      # O^T all heads
            wo = sbp.tile([P, 2, D], BF16, tag="wo")

            nc.gpsimd.load_library(library_config.attn)

            # zero rows of the padded K^T slots; ones column of vaug (DVE)
            nc.vector.memset(ktp[64:128, 0, :], 0.0)
            nc.vector.memset(ktp[0:64, 1, :], 0.0)
            nc.vector.memset(ktp[64:128, 2, :], 0.0)
            nc.vector.memset(ktp[0:64, 3, :], 0.0)
            nc.vector.memset(vaug[:, :, :, 64:65], 1.0)

            # psum tags: a0/a1 [128,1024] (2 banks each), o0..o3 (1 bank each)
            def psA(i):
                return ps.tile([P, IHW], F32, tag=f"a{i % 2}", name=f"psA{i % 2}")

            _ot_ctr = [0]

            def psO():
                i = _ot_ctr[0]
                _ot_ctr[0] += 1
                return ps.tile([P, 512], F32, tag=f"o{i % 4}", name=f"psO{i % 4}")

            # ------------- phase A: loads + projections (xt/w scoped) -------
            with tc.tile_pool(name="sbl", bufs=1) as sbl:
                wq = sbl.tile([P, CO, DSH], BF16, tag="wq")
                wk = sbl.tile([P, CO, DSH], BF16, tag="wk")
                wv = sbl.tile([P, CO, DSH], BF16, tag="wv")
                xt = [sbl.tile([P, NTOK], BF16, tag=f"xt{o}", name=f"xt{o}")
                      for o in range(CO)]
                # spread input loads over the two hardware DMA queues;
                # mt0 halves of Wq/Wk first so A1 can start immediately
                nc.scalar.dma_start(wq[:, :, 0:P], wq_t[:, :, 0:P])
                nc.sync.dma_start(wk[:, :, 0:P], wk_t[:, :, 0:P])
                for o in range(CO):
                    eng = nc.sync if o % 2 == 0 else nc.scalar
                    eng.dma_start(xt[o][:], xt_t[:, o, :])
                nc.scalar.dma_start(wq[:, :, P:DSH], wq_t[:, :, P:DSH])
                nc.sync.dma_start(wk[:, :, P:DSH], wk_t[:, :, P:DSH])
                nc.scalar.dma_start(wv[:], wv_t)
                nc.sync.dma_start(wo[:], wo_t)

                # A1: Q(mt0) + K(mt0), o-streamed so the PE starts on the
                # first xT chunk.  Q groups [128,1024] on a-tags; K groups
                # [128,512] per (ih,c) on o-tags.
                ppq = {ih: psA(ih) for ih in range(NIH)}
                ppk = {(ih, c): psO() for ih in range(NIH) for c in range(NC2)}
                for o in range(CO):
                    for ih in range(NIH):
                        for c in range(NC2):
                            nc.tensor.matmul(
                                ppq[ih][:, c * 512:(c + 1) * 512],
                                wq[:, o, 0:P],
                                xt[o][:, ih * IHW + c * 512:
                                      ih * IHW + (c + 1) * 512],
                                start=(o == 0), stop=(o == CO - 1),
                            )
                            nc.tensor.matmul(
                                ppk[(ih, c)][:],
                                wk[:, o, 0:P],
                                xt[o][:, ih * IHW + c * 512:
                                      ih * IHW + (c + 1) * 512],
                                start=(o == 0), stop=(o == CO - 1),
                            )
                for ih in range(NIH):
                    sl = slice(ih * IHW, (ih + 1) * IHW)
                    nc.scalar.copy(qt[:, 0, sl], ppq[ih][:, :])
                    for c in range(NC2):
                        slc = slice(ih * IHW + c * 512, ih * IHW + (c + 1) * 512)
                        nc.scalar.copy(ktp[0:64, 0, slc], ppk[(ih, c)][0:64, :])
                        nc.scalar.copy(ktp[64:128, 1, slc], ppk[(ih, c)][64:128, :])

                # A2: Q(mt1) / K(mt1) (all xT resident by now), then V
                for w_sb, is_q in ((wq, True), (wk, False)):
                    for ih in range(NIH):
                        pp = psA(ih)
                        for c in range(NC2):
                            for o in range(CO):
                                nc.tensor.matmul(
                                    pp[:, c * 512:(c + 1) * 512],
                                    w_sb[:, o, P:2 * P],
                                    xt[o][:, ih * IHW + c * 512:
                                          ih * IHW + (c + 1) * 512],
                                    start=(o == 0), stop=(o == CO - 1),
                                )
                        sl = slice(ih * IHW, (ih + 1) * IHW)
                        if is_q:
                            nc.scalar.copy(qt[:, 1, sl], pp[:, :])
                        else:
                            nc.scalar.copy(ktp[0:64, 2, sl], pp[0:64, :])
                            nc.scalar.copy(ktp[64:128, 3, sl], pp[64:128, :])

                # V projection
                for it in range(NJB):
                    pv = psO()
                    for o in range(CO):
                        nc.tensor.matmul(
                            pv[:, 0:DSH],
                            xt[o][:, it * P:(it + 1) * P],
                            wv[:, o, :],
                            start=(o == 0), stop=(o == CO - 1),
                        )
                    nc.scalar.copy(
                        vaug[:, it, :, 0:64],
                        pv[:, 0:DSH].rearrange("p (h d) -> p h d", d=HD),
                    )

            # ------------- phase B: attention + output projection -----------
            with tc.tile_pool(name="sbw", bufs=1) as sbw:
                ES_BUFS = 24

                def op_unit(ih, mt, c, o_list, dest, evac="v", pe=None):
                    """One output-projection unit: psum [128,512], matmuls
                    over o_list, evacuate bf16, DMA out."""
                    if pe is None:
                        pe = psO()
                    n_o = len(o_list)
                    for i, o in enumerate(o_list):
                        nc.tensor.matmul(
                            pe[:],
                            wo[:, o, mt * P:(mt + 1) * P],
                            ota[:, o, ih * IHW + c * 512:
                                ih * IHW + (c + 1) * 512],
                            start=(i == 0), stop=(i == n_o - 1),
                        )
                    stg = sbw.tile([P, 512], BF16, tag="stg", bufs=6)
                    if evac == "v":
                        nc.vector.tensor_copy(stg[:], pe[:])
                    else:
                        nc.scalar.copy(stg[:], pe[:])
                    if dest == 0:
                        eng = nc.sync if mt % 2 == 0 else nc.scalar
                        eng.dma_start(
                            out_t[:, mt, ih * IHW + c * 512:
                                  ih * IHW + (c + 1) * 512], stg[:])
                    else:
                        nc.scalar.dma_start(
                            out0_t[:, mt, c * 512:(c + 1) * 512], stg[:])

                def normalize(ih, pair, heads, pso, direct, z_on_act=False):
                    """Scale O^T rows by 1/Z.  direct=True multiplies PSUM
                    in place (shorter chain, holds the o-tag banks); False
                    evacuates first (frees banks for the next window's PV).
                    direct path runs fully c-major (z, recip, broadcast,
                    mult per c-half) so the c0 outputs unblock downstream
                    c-major consumers as early as possible.  z_on_act moves
                    the z-copies to the ACT engine — only safe for the last
                    window, where no exp stream follows in the ACT queue."""
                    rb, rts, zts, ot = {}, {}, {}, {}
                    for k, h in enumerate(heads):
                        zts[h] = sbw.tile([1, IHW], F32, tag="zt", bufs=2,
                                          name="zt")
                        rts[h] = sbw.tile([1, IHW], F32, tag="rt", bufs=2,
                                          name="rt")
                        rb[h] = sbw.tile([64, IHW], F32, tag="rb", bufs=2,
                                         name="rb")
                    if direct:
                        for c in range(NC2):
                            cs = slice(c * 512, (c + 1) * 512)
                            for k, h in enumerate(heads):
                                if z_on_act:
                                    nc.scalar.copy(zts[h][:, cs],
                                                   pso[(h, c)][64:65, :])
                                else:
                                    nc.vector.tensor_copy(
                                        zts[h][:, cs], pso[(h, c)][64:65, :])
                            for k, h in enumerate(heads):
                                nc.vector.reciprocal_approx_fast(
                                    out=rts[h][:, cs], in_=zts[h][:, cs])
                            for k, h in enumerate(heads):
                                nc.gpsimd.partition_broadcast(
                                    rb[h][:, cs], rts[h][:, cs])
                            for k, h in enumerate(heads):
                                row = slice((h % 2) * 64, (h % 2) * 64 + 64)
                                nc.vector.tensor_mul(
                                    ota[row, h // 2, ih * IHW + c * 512:
                                        ih * IHW + (c + 1) * 512],
                                    pso[(h, c)][0:64, :],
                                    rb[h][:, cs],
                                )
                        return
                    for k, h in enumerate(heads):
                        for c in range(NC2):
                            nc.vector.tensor_copy(
                                zts[h][:, c * 512:(c + 1) * 512],
                                pso[(h, c)][64:65, :])
                        nc.vector.reciprocal_approx_fast(out=rts[h][:],
                                                         in_=zts[h][:])
                        ot[h] = sbw.tile([64, IHW], F32, tag="otmp",
                                         bufs=2, name="otmp")
                        for c in range(NC2):
                            nc.vector.tensor_copy(
                                ot[h][:, c * 512:(c + 1) * 512],
                                pso[(h, c)][0:64, :])
                    for c in range(NC2):
                        for k, h in enumerate(heads):
                            nc.gpsimd.partition_broadcast(
                                rb[h][:, c * 512:(c + 1) * 512],
                                rts[h][:, c * 512:(c + 1) * 512])
                    for c in range(NC2):
                        for k, h in enumerate(heads):
                            row = slice((h % 2) * 64, (h % 2) * 64 + 64)
                            nc.vector.tensor_mul(
                                ota[row, h // 2, ih * IHW + c * 512:
                                    ih * IHW + (c + 1) * 512],
                                ot[h][:, c * 512:(c + 1) * 512],
                                rb[h][:, c * 512:(c + 1) * 512],
                            )

                for wi, (ih, pair) in enumerate([(0, 0), (0, 1), (1, 0), (1, 1)]):
                    heads = (2 * pair, 2 * pair + 1)
                    pso = {}

                    def get_pso(h, c):
                        if (h, c) not in pso:
                            pso[(h, c)] = psO()
                        return pso[(h, c)]

                    def emit_S(jb):
                        pst = {}
                        for k, h in enumerate(heads):
                            pa = psA(k)
                            pst[h] = pa
                            for c in range(NC2):
                                nc.tensor.matmul(
                                    pa[:, c * 512:(c + 1) * 512],
                                    ktp[:, h, jb * P:(jb + 1) * P],
                                    qt[:, h // 2, ih * IHW + c * 512:
                                       ih * IHW + (c + 1) * 512],
                                    start=True, stop=True,
                                )
                        return pst

                    def emit_exp(pst):
                        ess = {}
                        for k, h in enumerate(heads):
                            es = sbw.tile([P, IHW], BF16, tag="es",
                                          bufs=ES_BUFS)
                            nc.scalar.activation(
                                es[:], pst[h][:],
                                mybir.ActivationFunctionType.Exp,
                            )
                            ess[h] = es
                        return ess

                    def emit_PV(jb, ess):
                        for k, h in enumerate(heads):
                            for c in range(NC2):
                                nc.tensor.matmul(
                                    get_pso(h, c)[0:65, :],
                                    vaug[:, jb, h, 0:65],
                                    ess[h][:, c * 512:(c + 1) * 512],
                                    start=(jb == 0), stop=(jb == NJB - 1),
                                )

                    # per-jb schedules: (op_units_emitted_this_jb, pvs)
                    if wi < 2:
                        op_sched = {}
                        pv_sched = {jb: [jb - 1] for jb in range(1, NJB)}
                        pv_sched[NJB - 1] = [NJB - 2, NJB - 1]
                        pv_sched[NJB] = []
                    elif wi == 2:
                        # absorb OP(ih=0) full units at jb2..9 (after the
                        # previous window's normalize chain); PV deferred
                        units = [(0, mt, c, [0, 1], 0)
                                 for c in range(NC2) for mt in range(8)]
                        op_sched = {jb: units[2 * (jb - 2):2 * (jb - 2) + 2]
                                    for jb in range(2, 10)}
                        pv_sched = {10: [0, 1], 11: [2, 3], 12: [4, 5, 6],
                                    13: [7, 8, 9], 14: [10, 11, 12],
                                    15: [14, 15], NJB: []}
                        pv_sched[14] = [10, 11, 12, 13]
                    else:
                        # absorb OP(ih=1, o=0) units at jb2..9; PV back-loaded
                        units = [(1, mt, c, [0], 1)
                                 for c in range(NC2) for mt in range(8)]
                        op_sched = {jb: units[2 * (jb - 2):2 * (jb - 2) + 2]
                                    for jb in range(2, 10)}
                        pv_sched = {9: [0], 10: [1, 2], 11: [3, 4],
                                    12: [5, 6], 13: [7, 8, 9],
                                    14: [10, 11, 12, 13], 15: [14, 15],
                                    NJB: []}

                    ess_all = {}
                    for jb in range(NJB):
                        pst = emit_S(jb)
                        ess_all[jb] = emit_exp(pst)
                        for u in op_sched.get(jb, []):
                            op_unit(*u)
                        for pv_jb in pv_sched.get(jb, []):
                            emit_PV(pv_jb, ess_all[pv_jb])
                    for pv_jb in pv_sched.get(NJB, []):
                        emit_PV(pv_jb, ess_all[pv_jb])

                    normalize(ih, pair, heads, pso, direct=(wi >= 1),
                              z_on_act=(wi == 3))

                # tail: OP(ih=1, o=1) units, c-major so the c0 normalize
                # outputs unblock the first half; evacs alternate ACT/DVE;
                # psum rotates over 6 tags (the a-tags are free by now)
                tail_tags = ["o0", "o1", "o2", "o3", "a0", "a1"]
                for i, (c, mt) in enumerate(
                        [(c, mt) for c in range(NC2) for mt in range(8)]):
                    pe = ps.tile([P, 512], F32, tag=tail_tags[i % 6],
                                 name=f"psT{i % 6}")
                    op_unit(1, mt, c, [1], 0,
                            evac=("s" if mt % 2 else "v"), pe=pe)

    nc.compile()
    return nc


_NC_CACHE = None


def _get_nc():
    global _NC_CACHE
    if _NC_CACHE is None:
        _NC_CACHE = build_nc()
    return _NC_CACHE


def kernel(x, Wq, Wk, Wv, Wo, bo, _trace=False):
    x = np.asarray(x, dtype=DT_NP)
    Wq = np.asarray(Wq, dtype=DT_NP)
    Wk = np.asarray(Wk, dtype=DT_NP)
    Wv = np.asarray(Wv, dtype=DT_NP)
    Wo = np.asarray(Wo, dtype=DT_NP)
    bo = np.asarray(bo, dtype=DT_NP)
    B = x.shape[0]

    nc = _get_nc()
    in_maps = []
    for core in range(8):
        b, hg = divmod(core, 4)
        rows = slice(hg * DSH, (hg + 1) * DSH)
        in_maps.append({
            "xt": np.ascontiguousarray(x[b].T).astype(BF16_NP),
            "wqt": np.ascontiguousarray(Wq[rows, :].T).astype(BF16_NP),
            "wkt": np.ascontiguousarray((Wk[rows, :] * SCALE).T).astype(BF16_NP),
            "wvt": np.ascontiguousarray(Wv[rows, :].T).astype(BF16_NP),
            "wot": np.ascontiguousarray(Wo[:, rows].T).astype(BF16_NP),
        })

    res = bass_utils.run_bass_kernel_spmd(
        nc, in_maps, core_ids=list(range(8)), trace=_trace)

    out = np.zeros((B, NTOK, D), dtype=DT_NP)
    for core in range(8):
        b = core // 4
        out[b] += res.results[core]["outt"].T.astype(DT_NP)
        out[b, IHW:NTOK, :] += res.results[core]["out0"].T.astype(DT_NP)
    out += bo
    if _trace:
        kernel.last_results = res
    return out
